# revision 19
# baseline (speedup 1.0000x reference)
"""Multi-head causal attention (B=2, S=2048, D=1024, H=16, d=64) on 8 trn2 cores.

Sharding: core c -> batch b=c//4, head-group hg=c%4 (4 heads, 256 of 1024 dims).
Each core computes its 4 heads' attention + its partial out-projection; host
sums the 4 partials per batch and adds the bias.

x is transposed on the host (X^T in DRAM) so no device transposes are needed.
Input DMAs are emitted just-in-time on two HWDGE queues (sync + scalar;
scalar only before the exp era since a DMA costs ~0.7us of ACT sequencer).
The attention stream is one GLOBAL software pipeline across all (qc, head
pair) passes: AV matmuls lag the score matmuls by 2 slots so the PE never
waits on exp (ACT) or the causal-mask multiply (DVE); softmax normalization
is deferred to the following pass; projection / out-projection matmuls are
interleaved as PE fillers between slots. Scores use the transposed S^T[k,q]
layout with denominators from a ones-column appended to V.
"""
import sys

sys.path.insert(0, "/opt/trn_rl_repo")

import numpy as np
import ml_dtypes
import concourse.bass as bass
import concourse.mybir as mybir
from concourse import bacc
from concourse.tile import TileContext
from concourse.bass_utils import run_bass_kernel_spmd

F32 = mybir.dt.float32
BF16 = mybir.dt.bfloat16
AF = mybir.ActivationFunctionType
OP = mybir.AluOpType

S = 2048          # sequence length
D = 1024          # model dim
HD = 64           # head dim
NHL = 4           # heads per core
DL = 256          # local out dims (NHL * HD)
NQC = 4           # q chunks of 512
QW = 512          # q chunk width
NST = 16          # seq tiles of 128
NIC = 8           # input-dim chunks of 128
LAG = 2           # AV trails scores by this many slots


def build_bass():
    nc = bacc.Bacc("TRN2", target_bir_lowering=False, debug=False, num_devices=8)

    xt_d = nc.dram_tensor("xt", [D, S], BF16, kind="ExternalInput")
    wq_d = nc.dram_tensor("wq", [D, DL], BF16, kind="ExternalInput")
    wk_d = nc.dram_tensor("wk", [D, DL], BF16, kind="ExternalInput")
    wv_d = nc.dram_tensor("wv", [D, DL], BF16, kind="ExternalInput")
    wo_d = nc.dram_tensor("wo", [DL, D], BF16, kind="ExternalInput")
    mb_d = nc.dram_tensor("maskb", [128, 2, QW], BF16, kind="ExternalInput")
    out_d = nc.dram_tensor("out", [S, D], F32, kind="ExternalOutput")

    with TileContext(nc) as tc:
        with (
            tc.tile_pool(name="consts", bufs=1) as consts,
            tc.tile_pool(name="xtp", bufs=1) as xtp,
            tc.tile_pool(name="qk", bufs=1) as qkp,
            tc.tile_pool(name="vv", bufs=1) as vvp,
            tc.tile_pool(name="ctxn", bufs=1) as ctxnp,
            tc.tile_pool(name="ptp", bufs=6) as ptp,
            tc.tile_pool(name="recp", bufs=2) as recp,
            tc.tile_pool(name="rbp", bufs=4) as rbp,
            tc.tile_pool(name="outp", bufs=3) as outp,
            tc.tile_pool(name="psA", bufs=2, space="PSUM") as psA,
            tc.tile_pool(name="psS", bufs=2, space="PSUM") as psS,
            tc.tile_pool(name="psC", bufs=2, space="PSUM") as psC,
        ):
            # ---- SBUF tiles
            wq = consts.tile([128, NIC, DL], BF16, tag="wq")
            wk = consts.tile([128, NIC, DL], BF16, tag="wk")
            wv = consts.tile([128, NIC, DL], BF16, tag="wv")
            wo = consts.tile([128, 2, D], BF16, tag="wo")
            maskb = consts.tile([128, 2, QW], BF16, tag="maskb")
            xts = [
                xtp.tile([128, NIC, QW], BF16, tag=f"xt{c}", name=f"xt{c}")
                for c in range(NQC)
            ]
            xt0h = [
                xtp.tile([128, NIC, QW // 2], BF16, tag=f"xt0h{i}", name=f"xt0h{i}")
                for i in range(2)
            ]

            def x0sl(ic, lo, hi):
                # chunk-0 columns [lo:hi) out of the two half tiles
                if hi <= QW // 2:
                    return xt0h[0][:, ic, lo:hi]
                return xt0h[1][:, ic, lo - QW // 2 : hi - QW // 2]
            qt = qkp.tile([128, 2, S], BF16, tag="qt")
            kt = qkp.tile([128, 2, S], BF16, tag="kt")
            vextb = vvp.tile([128, NST, NHL, HD + 1], BF16, tag="vextb")
            ctxn = ctxnp.tile([128, 2, S], BF16, tag="ctxn")

            def xdma(eng, c, lo, hi):
                qsl = slice(c * QW, (c + 1) * QW)
                eng.dma_start(
                    out=xts[c][:, lo:hi, :],
                    in_=xt_d.ap()[128 * lo : 128 * hi, qsl].rearrange(
                        "(c p) s -> p c s", p=128
                    ),
                )

            # startup DMAs: only what the prelude + attention(0..1) need; later
            # chunks go through the gpsimd SWDGE stream (throttled by compute
            # progress) so they don't steal DMA-engine bandwidth from x0.
            nc.sync.dma_start(out=wq, in_=wq_d.ap().rearrange("(c p) n -> p c n", p=128))
            nc.scalar.dma_start(out=wk, in_=wk_d.ap().rearrange("(c p) n -> p c n", p=128))
            for i in range(2):
                hsl = slice(i * (QW // 2), (i + 1) * (QW // 2))
                nc.sync.dma_start(
                    out=xt0h[i][:, 0:4, :],
                    in_=xt_d.ap()[0:512, hsl].rearrange("(c p) s -> p c s", p=128),
                )
                nc.scalar.dma_start(
                    out=xt0h[i][:, 4:8, :],
                    in_=xt_d.ap()[512:1024, hsl].rearrange("(c p) s -> p c s", p=128),
                )
            nc.sync.dma_start(out=wv, in_=wv_d.ap().rearrange("(c p) n -> p c n", p=128))
            nc.scalar.dma_start(out=maskb, in_=mb_d.ap())
            # V_ext ones column (denominator trick), one strided memset
            nc.vector.memset(vextb[:, :, :, HD : HD + 1], 1.0)

            # ---- PE filler machinery. Out-projection fillers are deferred
            # to the late (exp-heavy) q-chunks where the PE would otherwise
            # idle waiting on ACT; projection fillers pop FIFO anytime.
            filler = []
            cur_qc = [0]

            cur_slot = [0]

            def pop_filler(n=1):
                for _ in range(n):
                    pick = None
                    for idx, (k, fn) in enumerate(filler):
                        if k[0] == "out":
                            if cur_qc[0] < min(k[1] + 2, NQC - 1):
                                continue
                        pick = idx
                        break
                    if pick is None:
                        return
                    filler.pop(pick)[1]()

            def drain(key):
                rest = []
                for k, fn in filler:
                    if key is None or k == key:
                        fn()
                    else:
                        rest.append((k, fn))
                filler[:] = rest

            def mk_qtkt(dst, wsb, p, qc):
                def go():
                    if qc == 0:
                        for i in range(2):
                            acc_t = psA.tile([128, QW], F32, tag="pa")
                            acc = acc_t[:, 0 : QW // 2]
                            for ic in range(NIC):
                                nc.tensor.matmul(
                                    acc,
                                    wsb[:, ic, 128 * p : 128 * (p + 1)],
                                    xt0h[i][:, ic, :],
                                    start=(ic == 0),
                                    stop=(ic == NIC - 1),
                                )
                            nc.vector.tensor_copy(
                                dst[:, p, i * (QW // 2) : (i + 1) * (QW // 2)], acc
                            )
                        return
                    acc = psA.tile([128, QW], F32, tag="pa")
                    for ic in range(NIC):
                        nc.tensor.matmul(
                            acc,
                            wsb[:, ic, 128 * p : 128 * (p + 1)],
                            xts[qc][:, ic, :],
                            start=(ic == 0),
                            stop=(ic == NIC - 1),
                        )
                    nc.vector.tensor_copy(dst[:, p, qc * QW : (qc + 1) * QW], acc)
                return go

            def mk_v(st):
                def go():
                    c, r = divmod(st, 4)
                    acc_t = psA.tile([128, QW], F32, tag="pa")
                    acc = acc_t[:, 0:DL]
                    for ic in range(NIC):
                        lhs = (
                            x0sl(ic, 128 * r, 128 * (r + 1))
                            if c == 0
                            else xts[c][:, ic, 128 * r : 128 * (r + 1)]
                        )
                        nc.tensor.matmul(
                            acc, lhs, wv[:, ic, :],
                            start=(ic == 0),
                            stop=(ic == NIC - 1),
                        )
                    nc.vector.tensor_copy(
                        vextb[:, st, :, 0:HD], acc.rearrange("p (h e) -> p h e", h=NHL)
                    )
                return go

            def mk_outproj(t, tail=False):
                def go():
                    tsl = slice(128 * t, 128 * (t + 1))
                    osb = outp.tile([128, D], F32, tag="osb")
                    for nh in range(2):
                        po = psA.tile([128, QW], F32, tag="pa")
                        nsl = slice(QW * nh, QW * (nh + 1))
                        nc.tensor.matmul(
                            po, ctxn[:, 0, tsl], wo[:, 0, nsl], start=True, stop=False
                        )
                        nc.tensor.matmul(
                            po, ctxn[:, 1, tsl], wo[:, 1, nsl], start=False, stop=True
                        )
                        if tail and nh == 1:
                            nc.scalar.copy(osb[:, nsl], po)
                        else:
                            nc.vector.tensor_copy(osb[:, nsl], po)
                    eng = nc.scalar if tail else nc.sync
                    eng.dma_start(out=out_d.ap()[tsl, :], in_=osb)
                return go

            # ---- global attention pipeline
            pend = []

            def av(it):
                pt_, kp_, j_ = it["pt"], it["kp"], it["j"]
                for h, ctx_t in ((0, it["ctxa"]), (1, it["ctxb"])):
                    if j_ is None:
                        nc.tensor.matmul(
                            ctx_t, vextb[:, kp_, 2 * it["p"] + h, :], pt_[:, h, :],
                            start=(kp_ == 0), stop=False,
                        )
                    else:
                        w = QW - 128 * j_
                        nc.tensor.matmul(
                            ctx_t[:, 128 * j_ : QW],
                            vextb[:, kp_, 2 * it["p"] + h, :],
                            pt_[:, h, 0:w],
                            start=(kp_ == 0), stop=it["lastslot"],
                        )

            def norm(it):
                qc, p = it["qc"], it["p"]
                qsl = slice(qc * QW, (qc + 1) * QW)
                for h, ctx_t in ((0, it["ctxa"]), (1, it["ctxb"])):
                    rec1 = recp.tile([1, QW], F32, tag="rec")
                    nc.vector.tensor_copy(rec1, ctx_t[HD : HD + 1, :])
                    rb = rbp.tile([HD, QW], F32, tag="rb")
                    nc.gpsimd.partition_broadcast(rb, rec1)
                    rec = rbp.tile([HD, QW], F32, tag="rb2")
                    nc.vector.reciprocal_approx_fast(rec, rb)
                    nc.vector.scalar_tensor_tensor(
                        out=ctxn[64 * h : 64 * h + 64, p, qsl],
                        in0=ctx_t[0:HD, :],
                        scalar=1.0,
                        in1=rec,
                        op0=OP.mult,
                        op1=OP.mult,
                    )

            def flush_one():
                it = pend.pop(0)
                av(it)
                if it["lastslot"]:
                    norm(it)
                    if it["p"] == 1:
                        tail = it["qc"] == NQC - 1
                        for t in range(4 * it["qc"], 4 * it["qc"] + 4):
                            filler.append(
                                (("out", it["qc"]), mk_outproj(t, tail=tail))
                            )

            # ---- prelude: everything attention(0, 0) needs, emitted directly
            mk_qtkt(qt, wq, 0, 0)()
            mk_qtkt(kt, wk, 0, 0)()
            for st in range(4):
                mk_v(st)()
            filler.append((("p1", 0), mk_qtkt(qt, wq, 1, 0)))
            filler.append((("p1", 0), mk_qtkt(kt, wk, 1, 0)))

            # ---- main loop
            for qc in range(NQC):
                cur_qc[0] = qc
                if qc > 0:
                    drain(("p0", qc))
                if qc != 0 and qc < NQC - 1:
                    # dispatched from the ACT stream -> executes only after the
                    # preceding exps, so these transfers don't steal DMA
                    # bandwidth from the startup-critical chunks
                    xdma(nc.scalar, qc + 1, 0, 4)
                    xdma(nc.scalar, qc + 1, 4, 8)
                if qc < NQC - 1:
                    filler.append((("p0", qc + 1), mk_qtkt(qt, wq, 0, qc + 1)))
                    filler.append((("p0", qc + 1), mk_qtkt(kt, wk, 0, qc + 1)))
                    filler.append((("p1", qc + 1), mk_qtkt(qt, wq, 1, qc + 1)))
                    filler.append((("p1", qc + 1), mk_qtkt(kt, wk, 1, qc + 1)))
                    for st in range(4 * (qc + 1), 4 * (qc + 1) + 4):
                        filler.append((("p0", qc + 1), mk_v(st)))
                for p in (0, 1):
                    if p == 1:
                        drain(("p1", qc))
                    qsl = slice(qc * QW, (qc + 1) * QW)
                    ctxa = psC.tile([HD + 1, QW], F32, tag="ctx")
                    ctxb = psC.tile([HD + 1, QW], F32, tag="ctx")
                    n_slots = 4 * qc + 4
                    for kp in range(n_slots):
                        cur_slot[0] = kp
                        if qc == 0 and p == 0 and kp == 1:
                            xdma(nc.scalar, 1, 0, 4)
                            xdma(nc.scalar, 1, 4, 8)
                            nc.scalar.dma_start(
                                out=wo,
                                in_=wo_d.ap().rearrange("(c p) n -> p c n", p=128),
                            )
                        diag = kp >= 4 * qc
                        st_t = psS.tile([128, 2, QW], F32, tag="st")
                        pt = ptp.tile([128, 2, QW], BF16, tag="pt")
                        ksl = slice(kp * 128, (kp + 1) * 128)
                        if not diag:
                            for h in (0, 1):
                                nc.tensor.matmul(
                                    st_t[:, h, :],
                                    kt[64 * h : 64 * h + 64, p, ksl],
                                    qt[64 * h : 64 * h + 64, p, qsl],
                                    start=True, stop=True,
                                    tile_position=(64 * h, 0),
                                )
                            nc.scalar.activation(pt, st_t, AF.Exp, scale=0.125)
                            j = None
                        else:
                            j = kp - 4 * qc
                            w = QW - 128 * j
                            qtr = slice(qc * QW + 128 * j, (qc + 1) * QW)
                            for h in (0, 1):
                                nc.tensor.matmul(
                                    st_t[:, h, 0:w],
                                    kt[64 * h : 64 * h + 64, p, ksl],
                                    qt[64 * h : 64 * h + 64, p, qtr],
                                    start=True, stop=True,
                                    tile_position=(64 * h, 0),
                                )
                            nc.scalar.activation(
                                pt[:, :, 0:w], st_t[:, :, 0:w], AF.Exp, scale=0.125
                            )
                            nc.vector.tensor_mul(
                                pt[:, :, 0:w], pt[:, :, 0:w], maskb[:, :, 0:w]
                            )
                        pend.append(dict(
                            pt=pt, kp=kp, j=j, qc=qc, p=p,
                            ctxa=ctxa, ctxb=ctxb,
                            lastslot=(kp == n_slots - 1),
                        ))
                        pop_filler(2 if kp < 2 else 1)
                        while len(pend) > LAG:
                            flush_one()
            while pend:
                flush_one()
            drain(None)

    nc.finalize()
    return nc


def _maskb():
    # multiplicative causal mask: 0 where q_local < kp_local, else 1
    m = np.ones((128, QW), dtype=np.float32)
    kp = np.arange(128)[:, None]
    q = np.arange(QW)[None, :]
    m[q < kp] = 0.0
    return np.repeat(m[:, None, :], 2, axis=1).astype(ml_dtypes.bfloat16)


def shard_inputs(x, Wq, Wk, Wv, Wo):
    x = np.asarray(x, dtype=np.float32)
    Wq = np.asarray(Wq, dtype=ml_dtypes.bfloat16)
    Wk = np.asarray(Wk, dtype=ml_dtypes.bfloat16)
    Wv = np.asarray(Wv, dtype=ml_dtypes.bfloat16)
    Wo = np.asarray(Wo, dtype=ml_dtypes.bfloat16)
    mb = _maskb()
    xt = [
        np.ascontiguousarray(x[b].T).astype(ml_dtypes.bfloat16) for b in range(2)
    ]
    in_maps = []
    for c in range(8):
        b, hg = divmod(c, 4)
        sl = slice(DL * hg, DL * (hg + 1))
        in_maps.append({
            "xt": xt[b],
            "wq": np.ascontiguousarray(Wq[:, sl]),
            "wk": np.ascontiguousarray(Wk[:, sl]),
            "wv": np.ascontiguousarray(Wv[:, sl]),
            "wo": np.ascontiguousarray(Wo[sl, :]),
            "maskb": mb,
        })
    return in_maps


def run(inputs, trace=False, **kwargs):
    """Build, run on 8 cores, and return (full_output, BassKernelResults)."""
    nc = build_bass()
    bo = np.asarray(inputs["bo"], dtype=np.float32)
    in_maps = shard_inputs(**{k: v for k, v in inputs.items() if k != "bo"})
    res = run_bass_kernel_spmd(
        nc, in_maps, core_ids=list(range(8)), trace=trace, **kwargs
    )
    parts = [r["out"] for r in res.results]
    out = np.empty((2, S, D), dtype=np.float32)
    for b in range(2):
        out[b] = parts[4 * b] + parts[4 * b + 1] + parts[4 * b + 2] + parts[4 * b + 3]
        out[b] += bo[None, :]
    return out, res


def kernel(x, Wq, Wk, Wv, Wo, bo):
    out, _ = run(dict(x=x, Wq=Wq, Wk=Wk, Wv=Wv, Wo=Wo, bo=bo))
    return out


# revision 20
# speedup vs baseline: 1.1771x; 1.1771x over previous
"""Multi-head causal attention (B=2, S=2048, D=1024, H=16, d=64) on 8 trn2 cores.

Sharding: core c -> batch b=c//4, head-group hg=c%4 (4 heads, 256 of 1024 dims).
Each core computes its 4 heads' attention + its partial out-projection; host
sums the 4 partials per batch and adds the bias.

x is transposed on the host (X^T in DRAM) so no device transposes are needed.
Input DMAs are emitted just-in-time on two HWDGE queues (sync + scalar;
scalar only before the exp era since a DMA costs ~0.7us of ACT sequencer).
The attention stream is one GLOBAL software pipeline across all (qc, head
pair) passes: AV matmuls lag the score matmuls by 2 slots so the PE never
waits on exp (ACT) or the causal-mask multiply (DVE); softmax normalization
is deferred to the following pass; projection / out-projection matmuls are
interleaved as PE fillers between slots. Scores use the transposed S^T[k,q]
layout with denominators from a ones-column appended to V.
"""
import sys

sys.path.insert(0, "/opt/trn_rl_repo")

import numpy as np
import ml_dtypes
import concourse.bass as bass
import concourse.mybir as mybir
from concourse import bacc
from concourse.tile import TileContext
from concourse.bass_utils import run_bass_kernel_spmd

F32 = mybir.dt.float32
BF16 = mybir.dt.bfloat16
AF = mybir.ActivationFunctionType
OP = mybir.AluOpType

S = 2048          # sequence length
D = 1024          # model dim
HD = 64           # head dim
NHL = 4           # heads per core
DL = 256          # local out dims (NHL * HD)
NQC = 4           # q chunks of 512
QW = 512          # q chunk width
NST = 16          # seq tiles of 128
NIC = 8           # input-dim chunks of 128
LAG = 2           # AV trails scores by this many slots


def build_bass():
    nc = bacc.Bacc("TRN2", target_bir_lowering=False, debug=False, num_devices=8)

    xt_d = nc.dram_tensor("xt", [D, S], BF16, kind="ExternalInput")
    wq_d = nc.dram_tensor("wq", [D, DL], BF16, kind="ExternalInput")
    wk_d = nc.dram_tensor("wk", [D, DL], BF16, kind="ExternalInput")
    wv_d = nc.dram_tensor("wv", [D, DL], BF16, kind="ExternalInput")
    wo_d = nc.dram_tensor("wo", [DL, D], BF16, kind="ExternalInput")
    mb_d = nc.dram_tensor("maskb", [128, 2, QW], BF16, kind="ExternalInput")
    out_d = nc.dram_tensor("out", [S, D], F32, kind="ExternalOutput")

    with TileContext(nc) as tc:
        with (
            tc.tile_pool(name="consts", bufs=1) as consts,
            tc.tile_pool(name="xtp", bufs=1) as xtp,
            tc.tile_pool(name="qk", bufs=1) as qkp,
            tc.tile_pool(name="vv", bufs=1) as vvp,
            tc.tile_pool(name="ctxn", bufs=1) as ctxnp,
            tc.tile_pool(name="ptp", bufs=6) as ptp,
            tc.tile_pool(name="recp", bufs=2) as recp,
            tc.tile_pool(name="rbp", bufs=4) as rbp,
            tc.tile_pool(name="outp", bufs=3) as outp,
            tc.tile_pool(name="psA", bufs=2, space="PSUM") as psA,
            tc.tile_pool(name="psS", bufs=2, space="PSUM") as psS,
            tc.tile_pool(name="psC", bufs=2, space="PSUM") as psC,
        ):
            # ---- SBUF tiles
            wq = consts.tile([128, NIC, DL], BF16, tag="wq")
            wk = consts.tile([128, NIC, DL], BF16, tag="wk")
            wv = consts.tile([128, NIC, DL], BF16, tag="wv")
            wo = consts.tile([128, 2, D], BF16, tag="wo")
            maskb = consts.tile([128, 2, QW], BF16, tag="maskb")
            xts = [
                xtp.tile([128, NIC, QW], BF16, tag=f"xt{c}", name=f"xt{c}")
                for c in range(NQC)
            ]
            qt = qkp.tile([128, 2, S], BF16, tag="qt")
            kt = qkp.tile([128, 2, S], BF16, tag="kt")
            vextb = vvp.tile([128, NST, NHL, HD + 1], BF16, tag="vextb")
            ctxn = ctxnp.tile([128, 2, S], BF16, tag="ctxn")

            def xdma(eng, c, lo, hi):
                qsl = slice(c * QW, (c + 1) * QW)
                eng.dma_start(
                    out=xts[c][:, lo:hi, :],
                    in_=xt_d.ap()[128 * lo : 128 * hi, qsl].rearrange(
                        "(c p) s -> p c s", p=128
                    ),
                )

            # startup DMAs: only what the prelude + attention(0..1) need; later
            # chunks go through the gpsimd SWDGE stream (throttled by compute
            # progress) so they don't steal DMA-engine bandwidth from x0.
            nc.sync.dma_start(out=wq, in_=wq_d.ap().rearrange("(c p) n -> p c n", p=128))
            nc.scalar.dma_start(out=wk, in_=wk_d.ap().rearrange("(c p) n -> p c n", p=128))
            xdma(nc.sync, 0, 0, 4)
            xdma(nc.scalar, 0, 4, 8)
            nc.sync.dma_start(out=wv, in_=wv_d.ap().rearrange("(c p) n -> p c n", p=128))
            nc.scalar.dma_start(out=maskb, in_=mb_d.ap())
            # V_ext ones column (denominator trick), one strided memset
            nc.vector.memset(vextb[:, :, :, HD : HD + 1], 1.0)

            # ---- PE filler machinery. Out-projection fillers are deferred
            # to the late (exp-heavy) q-chunks where the PE would otherwise
            # idle waiting on ACT; projection fillers pop FIFO anytime.
            filler = []
            cur_qc = [0]

            cur_slot = [0]

            def pop_filler(n=1):
                for _ in range(n):
                    pick = None
                    for idx, (k, fn) in enumerate(filler):
                        if k[0] == "out":
                            if cur_qc[0] < min(k[1] + 2, NQC - 1):
                                continue
                        pick = idx
                        break
                    if pick is None:
                        return
                    filler.pop(pick)[1]()

            def drain(key):
                rest = []
                for k, fn in filler:
                    if key is None or k == key:
                        fn()
                    else:
                        rest.append((k, fn))
                filler[:] = rest

            def mk_qtkt(dst, wsb, p, qc):
                def go():
                    acc = psA.tile([128, QW], F32, tag="pa")
                    for ic in range(NIC):
                        nc.tensor.matmul(
                            acc,
                            wsb[:, ic, 128 * p : 128 * (p + 1)],
                            xts[qc][:, ic, :],
                            start=(ic == 0),
                            stop=(ic == NIC - 1),
                        )
                    nc.vector.tensor_copy(dst[:, p, qc * QW : (qc + 1) * QW], acc)
                return go

            def mk_v(st):
                def go():
                    c, r = divmod(st, 4)
                    acc_t = psA.tile([128, QW], F32, tag="pa")
                    acc = acc_t[:, 0:DL]
                    for ic in range(NIC):
                        nc.tensor.matmul(
                            acc,
                            xts[c][:, ic, 128 * r : 128 * (r + 1)],
                            wv[:, ic, :],
                            start=(ic == 0),
                            stop=(ic == NIC - 1),
                        )
                    nc.vector.tensor_copy(
                        vextb[:, st, :, 0:HD], acc.rearrange("p (h e) -> p h e", h=NHL)
                    )
                return go

            def mk_outproj(t, tail=False):
                def go():
                    tsl = slice(128 * t, 128 * (t + 1))
                    osb = outp.tile([128, D], F32, tag="osb")
                    for nh in range(2):
                        po = psA.tile([128, QW], F32, tag="pa")
                        nsl = slice(QW * nh, QW * (nh + 1))
                        nc.tensor.matmul(
                            po, ctxn[:, 0, tsl], wo[:, 0, nsl], start=True, stop=False
                        )
                        nc.tensor.matmul(
                            po, ctxn[:, 1, tsl], wo[:, 1, nsl], start=False, stop=True
                        )
                        if tail and nh == 1:
                            nc.scalar.copy(osb[:, nsl], po)
                        else:
                            nc.vector.tensor_copy(osb[:, nsl], po)
                    eng = nc.scalar if tail else nc.sync
                    eng.dma_start(out=out_d.ap()[tsl, :], in_=osb)
                return go

            # ---- global attention pipeline
            pend = []

            def av(it):
                pt_, kp_, j_ = it["pt"], it["kp"], it["j"]
                for h, ctx_t in ((0, it["ctxa"]), (1, it["ctxb"])):
                    if j_ is None:
                        nc.tensor.matmul(
                            ctx_t, vextb[:, kp_, 2 * it["p"] + h, :], pt_[:, h, :],
                            start=(kp_ == 0), stop=False,
                        )
                    else:
                        w = QW - 128 * j_
                        nc.tensor.matmul(
                            ctx_t[:, 128 * j_ : QW],
                            vextb[:, kp_, 2 * it["p"] + h, :],
                            pt_[:, h, 0:w],
                            start=(kp_ == 0), stop=it["lastslot"],
                        )

            def norm(it):
                qc, p = it["qc"], it["p"]
                qsl = slice(qc * QW, (qc + 1) * QW)
                for h, ctx_t in ((0, it["ctxa"]), (1, it["ctxb"])):
                    rec1 = recp.tile([1, QW], F32, tag="rec")
                    nc.vector.tensor_copy(rec1, ctx_t[HD : HD + 1, :])
                    rb = rbp.tile([HD, QW], F32, tag="rb")
                    nc.gpsimd.partition_broadcast(rb, rec1)
                    rec = rbp.tile([HD, QW], F32, tag="rb2")
                    nc.vector.reciprocal_approx_fast(rec, rb)
                    nc.vector.scalar_tensor_tensor(
                        out=ctxn[64 * h : 64 * h + 64, p, qsl],
                        in0=ctx_t[0:HD, :],
                        scalar=1.0,
                        in1=rec,
                        op0=OP.mult,
                        op1=OP.mult,
                    )

            def flush_one():
                it = pend.pop(0)
                av(it)
                if it["lastslot"]:
                    norm(it)
                    if it["p"] == 1:
                        tail = it["qc"] == NQC - 1
                        for t in range(4 * it["qc"], 4 * it["qc"] + 4):
                            filler.append(
                                (("out", it["qc"]), mk_outproj(t, tail=tail))
                            )

            # ---- prelude: everything attention(0, 0) needs, emitted directly
            mk_qtkt(qt, wq, 0, 0)()
            mk_qtkt(kt, wk, 0, 0)()
            for st in range(4):
                mk_v(st)()
            filler.append((("p1", 0), mk_qtkt(qt, wq, 1, 0)))
            filler.append((("p1", 0), mk_qtkt(kt, wk, 1, 0)))

            # ---- main loop
            for qc in range(NQC):
                cur_qc[0] = qc
                if qc > 0:
                    drain(("p0", qc))
                if qc != 0 and qc < NQC - 1:
                    # dispatched from the ACT stream -> executes only after the
                    # preceding exps, so these transfers don't steal DMA
                    # bandwidth from the startup-critical chunks
                    xdma(nc.scalar, qc + 1, 0, 4)
                    xdma(nc.scalar, qc + 1, 4, 8)
                if qc < NQC - 1:
                    filler.append((("p0", qc + 1), mk_qtkt(qt, wq, 0, qc + 1)))
                    filler.append((("p0", qc + 1), mk_qtkt(kt, wk, 0, qc + 1)))
                    filler.append((("p1", qc + 1), mk_qtkt(qt, wq, 1, qc + 1)))
                    filler.append((("p1", qc + 1), mk_qtkt(kt, wk, 1, qc + 1)))
                    for st in range(4 * (qc + 1), 4 * (qc + 1) + 4):
                        filler.append((("p0", qc + 1), mk_v(st)))
                for p in (0, 1):
                    if p == 1:
                        drain(("p1", qc))
                    qsl = slice(qc * QW, (qc + 1) * QW)
                    ctxa = psC.tile([HD + 1, QW], F32, tag="ctx")
                    ctxb = psC.tile([HD + 1, QW], F32, tag="ctx")
                    n_slots = 4 * qc + 4
                    for kp in range(n_slots):
                        cur_slot[0] = kp
                        if qc == 0 and p == 0 and kp == 1:
                            xdma(nc.scalar, 1, 0, 4)
                            xdma(nc.scalar, 1, 4, 8)
                            nc.scalar.dma_start(
                                out=wo,
                                in_=wo_d.ap().rearrange("(c p) n -> p c n", p=128),
                            )
                        diag = kp >= 4 * qc
                        st_t = psS.tile([128, 2, QW], F32, tag="st")
                        pt = ptp.tile([128, 2, QW], BF16, tag="pt")
                        ksl = slice(kp * 128, (kp + 1) * 128)
                        if not diag:
                            for h in (0, 1):
                                nc.tensor.matmul(
                                    st_t[:, h, :],
                                    kt[64 * h : 64 * h + 64, p, ksl],
                                    qt[64 * h : 64 * h + 64, p, qsl],
                                    start=True, stop=True,
                                    tile_position=(64 * h, 0),
                                )
                            nc.scalar.activation(pt, st_t, AF.Exp, scale=0.125)
                            j = None
                        else:
                            j = kp - 4 * qc
                            w = QW - 128 * j
                            qtr = slice(qc * QW + 128 * j, (qc + 1) * QW)
                            for h in (0, 1):
                                nc.tensor.matmul(
                                    st_t[:, h, 0:w],
                                    kt[64 * h : 64 * h + 64, p, ksl],
                                    qt[64 * h : 64 * h + 64, p, qtr],
                                    start=True, stop=True,
                                    tile_position=(64 * h, 0),
                                )
                            nc.scalar.activation(
                                pt[:, :, 0:w], st_t[:, :, 0:w], AF.Exp, scale=0.125
                            )
                            nc.vector.tensor_mul(
                                pt[:, :, 0:w], pt[:, :, 0:w], maskb[:, :, 0:w]
                            )
                        pend.append(dict(
                            pt=pt, kp=kp, j=j, qc=qc, p=p,
                            ctxa=ctxa, ctxb=ctxb,
                            lastslot=(kp == n_slots - 1),
                        ))
                        pop_filler(2 if kp < 2 else 1)
                        while len(pend) > LAG:
                            flush_one()
            while pend:
                flush_one()
            drain(None)

    nc.finalize()
    return nc


def _maskb():
    # multiplicative causal mask: 0 where q_local < kp_local, else 1
    m = np.ones((128, QW), dtype=np.float32)
    kp = np.arange(128)[:, None]
    q = np.arange(QW)[None, :]
    m[q < kp] = 0.0
    return np.repeat(m[:, None, :], 2, axis=1).astype(ml_dtypes.bfloat16)


def shard_inputs(x, Wq, Wk, Wv, Wo):
    x = np.asarray(x, dtype=np.float32)
    Wq = np.asarray(Wq, dtype=ml_dtypes.bfloat16)
    Wk = np.asarray(Wk, dtype=ml_dtypes.bfloat16)
    Wv = np.asarray(Wv, dtype=ml_dtypes.bfloat16)
    Wo = np.asarray(Wo, dtype=ml_dtypes.bfloat16)
    mb = _maskb()
    xt = [
        np.ascontiguousarray(x[b].T).astype(ml_dtypes.bfloat16) for b in range(2)
    ]
    in_maps = []
    for c in range(8):
        b, hg = divmod(c, 4)
        sl = slice(DL * hg, DL * (hg + 1))
        in_maps.append({
            "xt": xt[b],
            "wq": np.ascontiguousarray(Wq[:, sl]),
            "wk": np.ascontiguousarray(Wk[:, sl]),
            "wv": np.ascontiguousarray(Wv[:, sl]),
            "wo": np.ascontiguousarray(Wo[sl, :]),
            "maskb": mb,
        })
    return in_maps


def run(inputs, trace=False, **kwargs):
    """Build, run on 8 cores, and return (full_output, BassKernelResults)."""
    nc = build_bass()
    bo = np.asarray(inputs["bo"], dtype=np.float32)
    in_maps = shard_inputs(**{k: v for k, v in inputs.items() if k != "bo"})
    res = run_bass_kernel_spmd(
        nc, in_maps, core_ids=list(range(8)), trace=trace, **kwargs
    )
    parts = [r["out"] for r in res.results]
    out = np.empty((2, S, D), dtype=np.float32)
    for b in range(2):
        out[b] = parts[4 * b] + parts[4 * b + 1] + parts[4 * b + 2] + parts[4 * b + 3]
        out[b] += bo[None, :]
    return out, res


def kernel(x, Wq, Wk, Wv, Wo, bo):
    out, _ = run(dict(x=x, Wq=Wq, Wk=Wk, Wv=Wv, Wo=Wo, bo=bo))
    return out
